# revision 20
# baseline (speedup 1.0000x reference)
"""Trainium2 Bass kernel for nn_BERTEmbedding_65274912964883.

out[b, l, :] = token_table[seq[b, l]]
             + mean_{g in genres(seq[b, l])} genre_table[g]
             + pos_table[l]

Strategy (8 NeuronCores, SPMD, no collectives):
  - Data-parallel over batch: 256 sequences -> 32 per core (6400 tokens/core).
  - One combined bf16 table [VOCAB, 160] replicated per core: cols 0..127
    token embedding, 128..148 the normalized genre histogram
    (hist[v, g] = count(g)/n_genres(v), a host-side dense re-encoding of the
    ragged genre lists), rest zero. The genre mean reduces on device as
    hist_row @ genre_table per token.
  - The gather is the hard floor: HW indirect DMA fetches exactly one table
    row per partition per instruction (~1.43us of GpSimd cadence; verified
    by on-HW unit test - a multi-column offset AP silently degrades to
    "one offset per partition + contiguous rows"), so 6400 tokens = 50
    instructions ~= 72us. Everything else rides UNDER that stream, with
    consumers (PE transposes, DVE adds) executing right behind each gather
    so the Tile DMA-semaphore-lane reuse guards never stall the gather
    queue. Row stride must be a 16-byte multiple (304B here).
  - Per subtile: PE transpose of the gathered hist window [128, 21] ->
    [21, 128] (base partition 0 - nonzero bases produce NaN on HW), scalar
    PSUM->SBUF copy, then one K=21 matmul per subtile against the genre
    table writing into a per-4-subtile PSUM group.
  - token + positional terms are added on DVE (emb + posrot, then + PSUM),
    downcasting to bf16; positional rows come from a host-prebuilt rotated
    table (28 rotations). This replaces the baseline's on-device one-hot
    histogram build (~26us of DVE) and its PE identity-matmul adds.
  - Device writes output partition-major [128, NSUB, D] bf16; host
    un-permutes and upcasts to f32.
"""

import numpy as np
import ml_dtypes

import concourse.bacc as bacc
import concourse.mybir as mybir
import concourse.tile as tile
from concourse.bass import IndirectOffsetOnAxis
from concourse.bass_utils import run_bass_kernel_spmd

VOCAB = 100000
D = 128
G = 21          # genre ids are in [0, 20]
MAXG = 8
CW = 152        # combined-table row: 128 emb + 21 hist + 3 pad (bf16)
B, L = 256, 200
NCORES = 8
BC = B // NCORES          # sequences per core
N = BC * L                # tokens per core (6400)
SUB = 128                 # tokens per subtile (partition dim)
NSUB = N // SUB           # 50
GROUPS = [4] * 12 + [1, 1]   # subtiles per matmul/add group (sum = NSUB)
NROT = 25                 # distinct values of (128*i) % 200
NROTX = 28                # extended with 3 duplicates so groups never wrap

F32 = mybir.dt.float32
BF16 = mybir.dt.bfloat16
I32 = mybir.dt.int32

assert sum(GROUPS) == NSUB


def emit_core_kernel(tc, seq, ctab, gtab, posrot, ident, out):
    """Emit the per-core kernel into TileContext `tc`.

    seq    : DRAM [128, NSUB] int32, seq[p, i] = token id of token i*128+p
    ctab   : DRAM [VOCAB, CW] bf16 combined table (emb | hist | pad)
    gtab   : DRAM [G, D] bf16 genre table
    posrot : DRAM [128, NROTX*D] bf16, posrot[p, r*D+d] =
             pos_table[(128*r+p) % L, d]
    ident  : DRAM [128, 128] bf16 identity
    out    : DRAM [128, NSUB, D] bf16, out[p, i, :] = embedding of token
             i*128+p
    """
    nc = tc.nc
    add = mybir.AluOpType.add

    with (
        tc.tile_pool(name="const", bufs=1) as cpool,
        tc.tile_pool(name="work", bufs=2) as wpool,
        tc.tile_pool(name="psum", bufs=2, space="PSUM") as ppool,
    ):
        # seq first on the sync queue (first 8 columns, then the rest):
        # gathers depend only on it
        seq_sb = cpool.tile([128, NSUB], I32)
        nc.sync.dma_start(out=seq_sb[:, 0:8], in_=seq[:, 0:8])
        nc.sync.dma_start(out=seq_sb[:, 8:NSUB], in_=seq[:, 8:NSUB])
        # consts ride the scalar queue
        gtab_sb = cpool.tile([G, D], BF16)
        nc.scalar.dma_start(out=gtab_sb[:], in_=gtab)
        ident_sb = cpool.tile([128, 128], BF16)
        nc.scalar.dma_start(out=ident_sb[:], in_=ident)
        posrot_sb = cpool.tile([128, NROTX * D], BF16)
        nc.scalar.dma_start(out=posrot_sb[:], in_=posrot)

        # one persistent gather tile; writer and readers use per-subtile
        # column ranges so the region tracker proves disjointness and the
        # 50 gathers free-run back-to-back.
        cg_sb = cpool.tile([128, NSUB * CW], BF16)
        cg3 = cg_sb[:].rearrange("p (j c) -> p j c", c=CW)

        # the gather stream: one indirect DMA per 128-token subtile
        for j in range(NSUB):
            gi_ = nc.gpsimd.indirect_dma_start(
                out=cg_sb[:, j * CW:(j + 1) * CW],
                out_offset=None,
                in_=ctab,
                in_offset=IndirectOffsetOnAxis(
                    ap=seq_sb[:, j:j + 1], axis=0
                ),
            )
            # spread gathers over the 4 SWDGE rings so SDMA drain of one
            # ring does not backpressure descriptor generation
            qn = j % 4
            gi_.ins.queue = f"qPoolDynamic{qn if qn else ''}"

        # compute: per-subtile hist transpose + matmul, per-group adds
        i0 = 0
        for gi, ng in enumerate(GROUPS):
            gm_ps = ppool.tile([128, ng * D], F32, tag=f"gm{ng}", bufs=2)
            for j in range(ng):
                jj = i0 + j
                # histT: [128 tokens, 21] -> [21, 128] at base partition 0
                hT_ps = ppool.tile([G, 128], BF16, tag="hT", bufs=3)
                nc.tensor.transpose(
                    out=hT_ps[:],
                    in_=cg_sb[:, jj * CW + D:jj * CW + D + G],
                    identity=ident_sb[:],
                )
                hT_sb = wpool.tile([G, 128], BF16, tag="hTs", bufs=3)
                nc.scalar.copy(out=hT_sb[:], in_=hT_ps[:])
                # genre mean: PSUM[token, d] = histT.T @ gtab
                nc.tensor.matmul(
                    out=gm_ps[:, j * D:(j + 1) * D],
                    lhsT=hT_sb[:],
                    rhs=gtab_sb[:],
                    start=True, stop=True,
                    skip_group_check=True,
                )

            # emb + pos on DVE, then + genre mean (PSUM), downcast bf16
            r0 = i0 % NROT
            ep_sb = wpool.tile([128, ng * D], BF16, tag=f"ep{ng}", bufs=2)
            nc.vector.tensor_tensor(
                out=ep_sb[:].rearrange("p (j d) -> p j d", d=D),
                in0=cg3[:, i0:i0 + ng, 0:D],
                in1=posrot_sb[:, r0 * D:(r0 + ng) * D]
                    .rearrange("p (j d) -> p j d", d=D),
                op=add,
            )
            out_sb = wpool.tile([128, ng * D], BF16, tag=f"o{ng}", bufs=2)
            nc.vector.tensor_tensor(
                out=out_sb[:], in0=ep_sb[:], in1=gm_ps[:], op=add,
            )
            nc.sync.dma_start(
                out=out[:, i0:i0 + ng, :],
                in_=out_sb[:].rearrange("p (j d) -> p j d", d=D),
            )
            i0 += ng


def build_nc():
    nc = bacc.Bacc("TRN2", target_bir_lowering=False, debug=False,
                   num_swdge_queues=4)
    seq = nc.dram_tensor("seq", [128, NSUB], I32, kind="ExternalInput").ap()
    ctab = nc.dram_tensor("ctab", [VOCAB, CW], BF16, kind="ExternalInput").ap()
    gtab = nc.dram_tensor("gtab", [G, D], BF16, kind="ExternalInput").ap()
    posrot = nc.dram_tensor(
        "posrot", [128, NROTX * D], BF16, kind="ExternalInput").ap()
    ident = nc.dram_tensor("ident", [128, 128], BF16, kind="ExternalInput").ap()
    out = nc.dram_tensor("out", [128, NSUB, D], BF16,
                         kind="ExternalOutput").ap()

    with tile.TileContext(nc) as tc:
        emit_core_kernel(tc, seq, ctab, gtab, posrot, ident, out)
    nc.compile()
    return nc


_NC_CACHE = None


def _get_nc():
    global _NC_CACHE
    if _NC_CACHE is None:
        _NC_CACHE = build_nc()
    return _NC_CACHE


def make_ctab(token_table, token_genre_ids, genre_counts):
    gids = np.asarray(token_genre_ids).astype(np.int64)      # [V, MAXG]
    cnts = np.asarray(genre_counts).astype(np.int64)         # [V]
    valid = np.arange(MAXG)[None, :] < cnts[:, None]         # [V, MAXG]
    flat = (np.arange(VOCAB)[:, None] * G + gids)[valid]
    hist = np.bincount(flat, minlength=VOCAB * G).reshape(VOCAB, G)
    histn = hist.astype(np.float32) / cnts[:, None].astype(np.float32)

    ctab = np.zeros((VOCAB, CW), dtype=ml_dtypes.bfloat16)
    ctab[:, 0:D] = np.asarray(token_table, dtype=np.float32).astype(
        ml_dtypes.bfloat16)
    ctab[:, D:D + G] = histn.astype(ml_dtypes.bfloat16)
    return ctab


def make_posrot(pos_table):
    pos = np.asarray(pos_table, dtype=np.float32)
    pr = np.zeros((128, NROTX * D), dtype=np.float32)
    p = np.arange(128)
    for r in range(NROTX):
        pr[:, r * D:(r + 1) * D] = pos[(128 * r + p) % L, :]
    return pr.astype(ml_dtypes.bfloat16)


def prep_host_inputs(sequence, token_table, genre_table, pos_table,
                     token_genre_ids, genre_counts):
    """Host-side sharding / layout prep. Returns in_maps for the 8 cores."""
    seq = np.ascontiguousarray(np.asarray(sequence).astype(np.int32)).reshape(B, L)
    ctab = make_ctab(token_table, token_genre_ids, genre_counts)
    gtab = np.asarray(genre_table, dtype=np.float32)[:G].astype(
        ml_dtypes.bfloat16)
    posrot = make_posrot(pos_table)
    ident = np.eye(128, dtype=np.float32).astype(ml_dtypes.bfloat16)

    in_maps = []
    for c in range(NCORES):
        seq_core = seq[c * BC:(c + 1) * BC].reshape(N)
        # device layout: seq_dev[p, i] = seq_core[i*128 + p]
        seq_dev = np.ascontiguousarray(seq_core.reshape(NSUB, 128).T)
        in_maps.append({
            "seq": seq_dev,
            "ctab": ctab,
            "gtab": gtab,
            "posrot": posrot,
            "ident": ident,
        })
    return in_maps


def postprocess(results):
    """Un-permute per-core outputs, upcast to f32, concatenate to [B, L, D]."""
    outs = []
    for c in range(NCORES):
        o = np.asarray(results[c]["out"])  # [128, NSUB, D] = [p, i, d]
        outs.append(o.transpose(1, 0, 2).reshape(BC, L, D))
    return np.concatenate(outs, axis=0).astype(np.float32)


def kernel(sequence, token_table, genre_table, pos_table, token_genre_ids,
           genre_counts):
    nc = _get_nc()
    in_maps = prep_host_inputs(sequence, token_table, genre_table, pos_table,
                               token_genre_ids, genre_counts)
    res = run_bass_kernel_spmd(nc, in_maps, core_ids=list(range(NCORES)))
    return postprocess(res.results)
